# revision 1
# baseline (speedup 1.0000x reference)
"""DH-SFNN (dendritic two-layer spiking net + leaky readout) on 8 trn2
cores, pure batch data parallelism (16 rows/core).

Vs the previous version: fp8 DoubleRow matmuls for MM1/MM2/MMo (weights
x256-scaled to dodge fp8 subnormal flush; the 1/256 is folded into the
branch-sum bridge and the readout Exp scale), PE transposes for x ingest
(replacing 96 x ~1.2us DMA xbar transposes), t-major contiguous
membrane scan (3 small contiguous DVE ops per step instead of strided
ops + cross-engine hops; membrane stored negated so the threshold test
is a plain is_le STT), bf16 membrane history with bulk contiguous
spike extraction to fp8, and a DVE tensor_scalar bridge that does b-major->t-major
transposition + 1/256 descale + per-h bias add + bf16 cast in one op.
Readout uses an fp8 hi+lo split of Wo (exact to ~0.1%).
"""
import sys
sys.path.insert(0, "/opt/trn_rl_repo")

import numpy as np
import ml_dtypes

import concourse.bass as bass
from concourse import bacc, masks
import concourse.tile as tile
import concourse.mybir as mybir
from concourse.bass_utils import run_bass_kernel_spmd

F32 = mybir.dt.float32
BF16 = mybir.dt.bfloat16
FP8 = mybir.dt.float8e4
AL = mybir.AluOpType
ACTF = mybir.ActivationFunctionType
DR = mybir.MatmulPerfMode.DoubleRow

B, T, IN, H, OUT, BR = 128, 250, 700, 512, 20, 4
NCORES = 8
BL = B // NCORES          # 16
BT = BL * T               # 4000
INP = 768                 # padded input dim
HBR = H * BR              # 2048
KP1, KP2 = 3, 2           # fp8 k-pair count for MM1 (768) / MM2 (512)
NB = BL // 2              # 8 b-pair blocks
NCB = 500                 # 2 b * 250 t columns per block
HH = 4                    # h chunks
SCALE = 256.0             # fp8 weight scaling

_NC_CACHE = None


def _build_nc():
    nc = bacc.Bacc(None, target_bir_lowering=False)

    xin = nc.dram_tensor("xin", [BT, INP], F32, kind="ExternalInput")
    w1d = nc.dram_tensor("w1d", [KP1, 128, 2, HBR], FP8, kind="ExternalInput")
    w2d = nc.dram_tensor("w2d", [KP2, 128, 2, HBR], FP8, kind="ExternalInput")
    wohd = nc.dram_tensor("wohd", [HH, 128, OUT], FP8, kind="ExternalInput")
    wold = nc.dram_tensor("wold", [HH, 128, OUT], FP8, kind="ExternalInput")
    a1d = nc.dram_tensor("a1d", [128, HH * BL], F32, kind="ExternalInput")
    a2d = nc.dram_tensor("a2d", [128, HH * BL], F32, kind="ExternalInput")
    k1d = nc.dram_tensor("k1d", [128, HH], F32, kind="ExternalInput")
    k2d = nc.dram_tensor("k2d", [128, HH], F32, kind="ExternalInput")
    bc1d = nc.dram_tensor("bc1d", [128, HH * BR], F32, kind="ExternalInput")
    nk1d = nc.dram_tensor("nk1d", [128, HH * BR], F32, kind="ExternalInput")
    bc2d = nc.dram_tensor("bc2d", [128, HH * BR], F32, kind="ExternalInput")
    nk2d = nc.dram_tensor("nk2d", [128, HH * BR], F32, kind="ExternalInput")
    aocd = nc.dram_tensor("aocd", [OUT, 1], F32, kind="ExternalInput")
    nkod = nc.dram_tensor("nkod", [OUT, 1], F32, kind="ExternalInput")
    kod = nc.dram_tensor("kod", [OUT, 1], F32, kind="ExternalInput")
    tmd = nc.dram_tensor("tmd", [125, 2], F32, kind="ExternalInput")
    out_d = nc.dram_tensor("out", [1, BL * OUT], F32, kind="ExternalOutput")

    with tile.TileContext(nc) as tc:
        with (
            tc.tile_pool(name="const", bufs=1) as cpool,
            tc.tile_pool(name="xT", bufs=1) as xpool,
            tc.tile_pool(name="U", bufs=1) as upool,
            tc.tile_pool(name="M", bufs=1) as mpool,
            tc.tile_pool(name="S", bufs=1) as spool,
            tc.tile_pool(name="dsl", bufs=2) as dpool,
            tc.tile_pool(name="stage", bufs=3) as stage,
            tc.tile_pool(name="sm", bufs=2) as smpool,
            tc.tile_pool(name="pst", bufs=1, space=bass.MemorySpace.PSUM) as pst,
            tc.tile_pool(name="psa", bufs=4, space=bass.MemorySpace.PSUM) as psa,
            tc.tile_pool(name="psf", bufs=1, space=bass.MemorySpace.PSUM) as psf,
        ):
            # ---------------- constants ----------------
            w1s, w2s, wohs, wols = [], [], [], []
            for p in range(KP1):
                tl = cpool.tile([128, 2, HBR], FP8, tag=f"w1_{p}")
                nc.sync.dma_start(tl[:], w1d[p])
                w1s.append(tl)
            for p in range(KP2):
                tl = cpool.tile([128, 2, HBR], FP8, tag=f"w2_{p}")
                nc.sync.dma_start(tl[:], w2d[p])
                w2s.append(tl)
            for p in range(HH):
                tl = cpool.tile([128, OUT], FP8, tag=f"woh_{p}")
                nc.sync.dma_start(tl[:], wohd[p])
                wohs.append(tl)
                tl = cpool.tile([128, OUT], FP8, tag=f"wol_{p}")
                nc.sync.dma_start(tl[:], wold[p])
                wols.append(tl)
            a1 = cpool.tile([128, HH * BL], F32, tag="a1")
            a2 = cpool.tile([128, HH * BL], F32, tag="a2")
            k1 = cpool.tile([128, HH], F32, tag="k1")
            k2 = cpool.tile([128, HH], F32, tag="k2")
            bc1s = cpool.tile([128, HH * BR], F32, tag="bc1")
            nk1s = cpool.tile([128, HH * BR], F32, tag="nk1")
            bc2s = cpool.tile([128, HH * BR], F32, tag="bc2")
            nk2s = cpool.tile([128, HH * BR], F32, tag="nk2")
            aocs = cpool.tile([OUT, 1], F32, tag="aoc")
            nkos = cpool.tile([OUT, 1], F32, tag="nko")
            kos = cpool.tile([OUT, 1], F32, tag="ko")
            tm = cpool.tile([125, 2], F32, tag="tmask")
            for dst, src in ((a1, a1d), (a2, a2d), (k1, k1d), (k2, k2d),
                             (bc1s, bc1d), (nk1s, nk1d), (bc2s, bc2d),
                             (nk2s, nk2d), (aocs, aocd), (nkos, nkod),
                             (kos, kod), (tm, tmd)):
                nc.sync.dma_start(dst[:], src[:])
            identb = cpool.tile([128, 128], BF16, tag="identb")
            masks.make_identity(nc, identb[:])
            identf = cpool.tile([128, 128], F32, tag="identf")
            masks.make_identity(nc, identf[:])
            mzero = cpool.tile([128, HH * BL], BF16, tag="mzero")
            nc.vector.memset(mzero[:], 0.0)

            U = upool.tile([128, T, HH, BL], BF16, tag="U")
            M = mpool.tile([128, T, HH * BL], BF16, tag="M")
            S = spool.tile([128, T, HH, BL], FP8, tag="S")
            ybuf = cpool.tile([128, 2, HH * BL], F32, tag="ybuf")
            wbuf = cpool.tile([128, 2, HH * BL], BF16, tag="wbuf")

            # ---------------- x ingest: DMA + cast + PE transpose ----------
            xT = [xpool.tile([128, 2, BT], FP8, tag=f"xT_{p}", name=f"xT_{p}")
                  for p in range(KP1)]
            nrt = [(i, 128 if i < 31 else BT - 31 * 128) for i in range(32)]
            for i, rows in nrt:
                xs = stage.tile([128, INP], F32, tag="xs", name="xs")
                nc.sync.dma_start(xs[:rows, :], xin[i * 128:i * 128 + rows, :])
                xb = stage.tile([128, INP], BF16, tag="xb", name="xb")
                nc.scalar.copy(xb[:rows, :], xs[:rows, :])
                for k in range(2 * KP1):
                    ptr = pst.tile([128, 128], BF16, tag="xtr", name="xtr")
                    nc.tensor.transpose(ptr[:, :rows], xb[:rows,
                                        k * 128:(k + 1) * 128],
                                        identb[:rows, :rows])
                    nc.scalar.copy(xT[k // 2][:, k % 2, i * 128:i * 128 + rows],
                                   ptr[:, :rows])

            # ---------------- generic fp8 layer: MM + d-scan + bsum -> U ---
            def layer(w_tiles, nkp, rhs_fn, bcs, nks, ktile, atile):
                for hh in range(HH):
                    for nb in range(NB):
                        dts = []
                        for br in range(BR):
                            g = br * HH + hh
                            j = hh * BR + br
                            acc = psa.tile([128, NCB], F32, tag="acc",
                                           name="acc")
                            for p in range(nkp):
                                nc.tensor.matmul(
                                    acc[:],
                                    w_tiles[p][:, :, g * 128:(g + 1) * 128],
                                    rhs_fn(p, nb),
                                    start=(p == 0), stop=(p == nkp - 1),
                                    perf_mode=DR)
                            dt_ = dpool.tile([128, NCB], F32, tag=f"dt{br}",
                                             name=f"dt{br}")
                            for bl in range(2):
                                nc.vector.tensor_tensor_scan(
                                    dt_[:, bl * T:(bl + 1) * T],
                                    bcs[:, j:j + 1].broadcast_to([128, T]),
                                    acc[:, bl * T:(bl + 1) * T],
                                    nks[:, j:j + 1],
                                    op0=AL.mult, op1=AL.add)
                            dts.append(dt_)
                        t01 = dpool.tile([128, NCB], F32, tag="t01",
                                         name="t01")
                        nc.gpsimd.tensor_add(t01[:], dts[0][:], dts[1][:])
                        t23 = dpool.tile([128, NCB], F32, tag="t23",
                                         name="t23")
                        nc.gpsimd.tensor_add(t23[:], dts[2][:], dts[3][:])
                        s12 = dpool.tile([128, 2, T], F32, tag="s12",
                                         name="s12")
                        nc.vector.tensor_add(
                            s12[:].rearrange("p a b -> p (a b)"),
                            t01[:], t23[:])
                        # bridge to t-major U, x(1/256), +K, cast bf16
                        nc.vector.tensor_scalar(
                            U[:, :, hh, 2 * nb:2 * nb + 2],
                            s12[:].rearrange("p a b -> p b a"),
                            1.0 / SCALE, ktile[:, hh:hh + 1],
                            op0=AL.mult, op1=AL.add)

            # ---------------- membrane scan (t-major, contiguous) ----------
            # M holds the NEGATED membrane N = -m, so the spike test is
            # N <= -1 and the step is N[t] = a*N[t-1] + (s[t-1] - U[t]),
            # which maps onto a plain STT without the reverse1 flag.
            def mscan(atile):
                for t in range(T):
                    mprev = mzero[:] if t == 0 else M[:, t - 1]
                    tp = t % 2
                    nc.vector.scalar_tensor_tensor(
                        wbuf[:, tp], mprev, -1.0,
                        U[:, t].rearrange("p a b -> p (a b)"),
                        op0=AL.is_le, op1=AL.subtract)
                    nc.vector.tensor_tensor(ybuf[:, tp], mprev, atile[:],
                                            op=AL.mult)
                    nc.vector.tensor_tensor(M[:, t], ybuf[:, tp], wbuf[:, tp],
                                            op=AL.add)
                # bulk spike extraction -> fp8, split DVE/gpsimd
                Sf = S[:].rearrange("p t h b -> p (t h b)")
                Mf = M[:].rearrange("p t c -> p (t c)")
                Q = T * HH * BL // 4
                for q in range(4):
                    nc.vector.tensor_scalar(Sf[:, q * Q:(q + 1) * Q],
                                      Mf[:, q * Q:(q + 1) * Q],
                                      -1.0, None, op0=AL.is_le)

            # ---------------- run phases ----------------
            layer(w1s, KP1,
                  lambda p, nb: xT[p][:, :, nb * NCB:(nb + 1) * NCB],
                  bc1s, nk1s, k1, a1)
            mscan(a1)
            layer(w2s, KP2,
                  lambda p, nb: S[:, :, 2 * p:2 * p + 2,
                                  2 * nb:2 * nb + 2].rearrange(
                                      "p t h b -> p h b t"),
                  bc2s, nk2s, k2, a2)
            mscan(a2)

            # ---------------- readout ----------------
            acc_f = psf.tile([1, BL * OUT], F32, tag="accf")
            for nb in range(NB):
                po = psf.tile([OUT, NCB], F32, tag="po", name="po")
                mms = [(wohs, k) for k in range(HH)] + \
                      [(wols, k) for k in range(HH)]
                for mi, (wt, k) in enumerate(mms):
                    nc.tensor.matmul(
                        po[:], wt[k][:],
                        S[:, :, k, 2 * nb:2 * nb + 2].rearrange(
                            "p t b -> p b t"),
                        start=(mi == 0), stop=(mi == 7))
                mo = smpool.tile([OUT, NCB], F32, tag="mo", name="mo")
                for bl in range(2):
                    nc.vector.tensor_tensor_scan(
                        mo[:, bl * T:(bl + 1) * T],
                        aocs[:].broadcast_to([OUT, T]),
                        po[:, bl * T:(bl + 1) * T],
                        nkos[:],
                        op0=AL.mult, op1=AL.add)
                ex = smpool.tile([OUT, NCB], F32, tag="ex", name="ex")
                nc.scalar.activation(ex[:], mo[:], ACTF.Exp,
                                     bias=kos[:], scale=1.0 / SCALE)
                for bl in range(2):
                    b = 2 * nb + bl
                    for half in range(2):
                        ptr = pst.tile([125, OUT], F32, tag="trps",
                                       name="trps")
                        nc.tensor.transpose(
                            ptr[:], ex[:, bl * T + half * 125:
                                        bl * T + (half + 1) * 125],
                            identf[:OUT, :OUT])
                        et = smpool.tile([125, OUT], F32, tag="et", name="et")
                        nc.scalar.copy(et[:], ptr[:])
                        sm_sum = smpool.tile([125, 1], F32, tag="sms",
                                             name="sms")
                        nc.vector.reduce_sum(sm_sum[:], et[:],
                                             axis=mybir.AxisListType.X)
                        rc = smpool.tile([125, 1], F32, tag="rc", name="rc")
                        nc.vector.reciprocal(rc[:], sm_sum[:])
                        pr = smpool.tile([125, OUT], F32, tag="pr", name="pr")
                        nc.vector.tensor_scalar(
                            pr[:], et[:], rc[:], None, op0=AL.mult)
                        nc.tensor.matmul(
                            acc_f[:, b * OUT:(b + 1) * OUT],
                            tm[:, half:half + 1], pr[:],
                            start=(half == 0), stop=(half == 1))
            fin = smpool.tile([1, BL * OUT], F32, tag="fin", name="fin")
            nc.scalar.copy(fin[:], acc_f[:])
            nc.sync.dma_start(out_d[:], fin[:])

    nc.compile()
    return nc


def _sigmoid(x):
    return 1.0 / (1.0 + np.exp(-x.astype(np.float64)))


FP8NP = ml_dtypes.float8_e4m3


def _pair_tiles(Wt, npairs, ncols):
    """[128*2*npairs, ncols] -> [npairs, 128, 2, ncols] fp8 host layout."""
    return np.ascontiguousarray(
        Wt.reshape(npairs, 2, 128, ncols).transpose(0, 2, 1, 3)
    ).astype(FP8NP)


def _host_prep(inputs):
    f32 = np.float32
    x = np.asarray(inputs["x"], f32)
    W1 = np.asarray(inputs["W1"], f32); b1 = np.asarray(inputs["b1"], f32)
    W2 = np.asarray(inputs["W2"], f32); b2 = np.asarray(inputs["b2"], f32)
    Wo = np.asarray(inputs["Wo"], f32); bo = np.asarray(inputs["bo"], f32)
    mask1 = np.asarray(inputs["mask1"], f32)
    mask2 = np.asarray(inputs["mask2"], f32)
    beta1 = _sigmoid(np.asarray(inputs["tau_n1"], f32)).astype(f32)   # [H,BR]
    alpha1 = _sigmoid(np.asarray(inputs["tau_m1"], f32)).astype(f32)  # [H]
    beta2 = _sigmoid(np.asarray(inputs["tau_n2"], f32)).astype(f32)
    alpha2 = _sigmoid(np.asarray(inputs["tau_m2"], f32)).astype(f32)
    alpha_o = _sigmoid(np.asarray(inputs["tau_mo"], f32)).astype(f32)

    lam1 = ((1.0 - beta1) * (1.0 - alpha1)[:, None]).astype(f32)
    lam2 = ((1.0 - beta2) * (1.0 - alpha2)[:, None]).astype(f32)

    Wm1 = (W1 * mask1).reshape(H, BR, IN)
    W1p = (lam1[:, :, None] * Wm1).transpose(1, 0, 2).reshape(BR * H, IN)
    W1T = np.zeros((INP, HBR), f32)
    W1T[:IN, :] = W1p.T * SCALE
    w1d = _pair_tiles(W1T, KP1, HBR)

    Wm2 = (W2 * mask2).reshape(H, BR, H)
    W2p = (lam2[:, :, None] * Wm2).transpose(1, 0, 2).reshape(BR * H, H)
    W2T = np.ascontiguousarray(W2p.T) * SCALE
    w2d = _pair_tiles(W2T, KP2, HBR)

    WoT = np.ascontiguousarray(((1.0 - alpha_o)[:, None] * Wo).T) * SCALE
    woh = WoT.astype(FP8NP).astype(f32)
    wol = WoT - woh
    wohd = np.ascontiguousarray(woh.reshape(HH, 128, OUT)).astype(FP8NP)
    wold = np.ascontiguousarray(wol.reshape(HH, 128, OUT)).astype(FP8NP)

    b1r = b1.reshape(H, BR); b2r = b2.reshape(H, BR)
    K1br = ((1.0 - alpha1)[:, None] * b1r).astype(f32)                # [H,BR]
    K2br = ((1.0 - alpha2)[:, None] * b2r).astype(f32)
    K1 = K1br.sum(1); K2 = K2br.sum(1)

    a1d = np.broadcast_to(
        alpha1.reshape(HH, 128).T[:, :, None], (128, HH, BL)).reshape(
            128, HH * BL).astype(f32).copy()
    a2d = np.broadcast_to(
        alpha2.reshape(HH, 128).T[:, :, None], (128, HH, BL)).reshape(
            128, HH * BL).astype(f32).copy()
    k1d = np.ascontiguousarray(K1.reshape(HH, 128).T).astype(f32)
    k2d = np.ascontiguousarray(K2.reshape(HH, 128).T).astype(f32)

    bc1 = np.zeros((128, HH * BR), f32)
    nk1 = np.zeros((128, HH * BR), f32)
    bc2 = np.zeros((128, HH * BR), f32)
    nk2 = np.zeros((128, HH * BR), f32)
    for hh in range(HH):
        for br in range(BR):
            j = hh * BR + br
            bc1[:, j] = beta1[hh * 128:(hh + 1) * 128, br]
            nk1[:, j] = -SCALE * K1br[hh * 128:(hh + 1) * 128, br]
            bc2[:, j] = beta2[hh * 128:(hh + 1) * 128, br]
            nk2[:, j] = -SCALE * K2br[hh * 128:(hh + 1) * 128, br]
    aoc = alpha_o.reshape(OUT, 1).astype(f32)
    nko = (-SCALE * bo).reshape(OUT, 1).astype(f32)
    kod = bo.reshape(OUT, 1).astype(f32)
    tmask = np.zeros((125, 2), f32)
    tmask[11:, 0] = 1.0
    tmask[:, 1] = 1.0

    shared = dict(w1d=w1d, w2d=w2d, wohd=wohd, wold=wold, a1d=a1d, a2d=a2d,
                  k1d=k1d, k2d=k2d, bc1d=bc1, nk1d=nk1, bc2d=bc2, nk2d=nk2,
                  aocd=aoc, nkod=nko, kod=kod, tmd=tmask)
    xs = []
    for c in range(NCORES):
        xc = x[c * BL:(c + 1) * BL].reshape(BT, IN)
        xp = np.zeros((BT, INP), f32)
        xp[:, :IN] = xc
        xs.append(xp)
    return shared, xs


def kernel(**inputs):
    global _NC_CACHE
    if _NC_CACHE is None:
        _NC_CACHE = _build_nc()
    nc = _NC_CACHE
    shared, xs = _host_prep(inputs)
    in_maps = [dict(shared, xin=xs[c]) for c in range(NCORES)]
    res = run_bass_kernel_spmd(nc, in_maps, core_ids=list(range(NCORES)))
    out = np.concatenate(
        [res.results[c]["out"].reshape(BL, OUT) for c in range(NCORES)],
        axis=0)
    return out.astype(np.float32)

